# revision 10
# baseline (speedup 1.0000x reference)
"""Distributed triangle multiplication (AlphaFold-style) for 8 Trainium2 NeuronCores.

v2 design (per core, SPMD; host row-shards pair as bf16, 96 rows/core):
  Phase 1a: per 2048-pos block: bn stats (DVE), sqrt (Act, sqrt-set),
    normalize (Act Identity / DVE tensor_scalar split), DMA-crossbar
    transpose into a RESIDENT channel-major xT [128, POS] SBUF tile
    (no DRAM round trip; crossbar replaces all PE transposes).
  Phase 1b: per channel-group q in {0,1}: channel-major projection+gate
    matmuls (W stationary), batched sigmoid (Act, sigmoid-set),
    product (DVE scalar_tensor_tensor, folds c1 bias), contiguous
    stores into ab_i[q] [128 rows=(dest*16+u), POS].
  AllToAll #1 x2 (one per group, fired as soon as the group is done).
  Phase 2: per einsum channel: A/B operands assembled k-major directly
    from ab_o via DMA-crossbar transposes (DRAM->SBUF), batched 768^3
    bf16 matmuls, PSUM->SBUF copies split Act/DVE, row-split stores to
    o2_i [128 rows=(dest*16+ch), POS].
  AllToAll #2 (one buffer; row r = einsum channel r for my positions).
  Phase 3 passA: per block: single-DMA load o2T [128ch, 2048], crossbar
    to position-major, bn stats; one batched sqrt at the end.
  Phase 3 passB: reload o2T, crossbar, normalize (Act/DVE), crossbar
    back to channel-major, out/gating matmuls vs resident consts and
    resident xT, batched sigmoid, fused (out+cf)*sig (DVE), f32 store
    channel-major; host transposes to [N, N, C].
Activation table sets never interleave sqrt with sigmoid -> ~4 table
loads total instead of 142.
"""
import sys

for _p in ("/opt/trn_rl_repo", "/opt/trn_rl_repo/concourse"):
    if _p not in sys.path:
        sys.path.insert(0, _p)

import os
import numpy as np
import ml_dtypes

import concourse.bass as bass
import concourse.tile as tile
from concourse import bacc, mybir
from concourse.bass_utils import run_bass_kernel_spmd

F32 = mybir.dt.float32
BF16 = mybir.dt.bfloat16
AF = mybir.ActivationFunctionType
ALU = mybir.AluOpType

N = 768
C = 128
NCORES = 8
R = N // NCORES            # 96 rows per core
POS = R * N                # 73728 positions per core
EPS = 1e-5
GS = 16                    # 16 x 128-pos subtiles per block
NG = POS // (GS * 128)     # 36 blocks of 2048 positions
NORM_DVE_J = 3             # j-subtiles normalized on DVE (rest on Act)

_PROGRAM_CACHE = {}
LAST_EXEC_NS = None
LAST_TRACE = None


def _build_program(use_mask, sim=False, dbg=False):
    nc = bacc.Bacc("TRN2", target_bir_lowering=False, debug=False,
                   num_devices=1 if sim else NCORES)

    def _collective(ins, outs):
        if sim:
            nc.sync.dma_start(out=outs[0], in_=ins[0])
        else:
            nc.gpsimd.collective_compute("AllToAll", ALU.bypass,
                                         [list(range(NCORES))],
                                         ins=ins, outs=outs)

    pair_r = nc.dram_tensor("pair_r", [POS, C], BF16, kind="ExternalInput").ap()
    # cols: q*256 + kind*128 + p   (kind 0=proj, 1=gate)
    w1t = nc.dram_tensor("w1t", [C, 512], BF16, kind="ExternalInput").ap()
    # wf: [c, 2C]: cols 0:C out_w (ln2_w folded), C:2C gating_w (ln1_w folded)
    wfin = nc.dram_tensor("wfin", [C, 2 * C], BF16, kind="ExternalInput").ap()
    # biases: [1, 512] c1 (q,kind,p order); [1, 256] cfin (out|gating)
    c1r = nc.dram_tensor("c1r", [1, 512], F32, kind="ExternalInput").ap()
    cfr = nc.dram_tensor("cfr", [1, 256], F32, kind="ExternalInput").ap()
    if use_mask:
        mask_r = nc.dram_tensor("mask_r", [POS // 1024, 1024], F32,
                                kind="ExternalInput").ap()

    ab_i = [nc.dram_tensor(f"ab{q}_i", [128, POS], BF16).ap() for q in range(2)]
    ab_o = [nc.dram_tensor(f"ab{q}_o", [128, POS], BF16).ap() for q in range(2)]
    o2_i = nc.dram_tensor("o2_i", [128, POS], BF16).ap()
    o2_o = nc.dram_tensor("o2_o", [128, POS], BF16).ap()
    out_r = nc.dram_tensor("out_r", [C, POS], F32, kind="ExternalOutput").ap()
    if dbg:
        dbg_xT = nc.dram_tensor("dbg_xT", [C, POS], BF16,
                                kind="ExternalOutput").ap()
        dbg_ab0i = nc.dram_tensor("dbg_ab0i", [128, POS], BF16,
                                  kind="ExternalOutput").ap()
        dbg_o2i = nc.dram_tensor("dbg_o2i", [128, POS], BF16,
                                 kind="ExternalOutput").ap()

    with tile.TileContext(nc) as tc:
        with tc.tile_pool(name="consts", bufs=1) as cpool:
            w1sb = cpool.tile([C, 512], BF16)
            nc.sync.dma_start(w1sb[:], w1t[:, :])
            wfsb = cpool.tile([C, 2 * C], BF16)
            nc.sync.dma_start(wfsb[:], wfin[:, :])
            c1sb = cpool.tile([128, 4], F32)   # [p, q*2+kind]
            nc.sync.dma_start(
                c1sb[:], c1r[0, :].rearrange("(a p) -> p a", p=128))
            cfsb = cpool.tile([128, 2], F32)   # [oc, 0=out 1=gating]
            nc.sync.dma_start(
                cfsb[:], cfr[0, :].rearrange("(a p) -> p a", p=128))
            epsb = cpool.tile([128, 1], F32)
            nc.vector.memset(epsb[:], EPS)
            # resident normalized channel-major x^T
            xT = cpool.tile([C, POS], BF16)

            # ================= Phase 1a: LN1 + transpose =================
            with tc.tile_pool(name="p1x", bufs=3) as p1x, \
                 tc.tile_pool(name="p1st", bufs=2 * GS + 4) as p1st, \
                 tc.tile_pool(name="p1mv", bufs=3) as p1mv, \
                 tc.tile_pool(name="p1xn", bufs=3) as p1xn:
                for g in range(NG):
                    xt8 = p1x.tile([128, GS, C], BF16, tag="xt8")
                    nc.sync.dma_start(
                        xt8[:],
                        pair_r[g * GS * 128:(g + 1) * GS * 128, :].rearrange(
                            "(s p) c -> p s c", p=128))
                    mv = p1mv.tile([128, 2 * GS], F32, tag="mv")
                    for j in range(GS):
                        st6 = p1st.tile([128, 6], F32, tag="st6")
                        nc.vector.bn_stats(st6[:], xt8[:, j, :])
                        nc.vector.bn_aggr(mv[:, 2 * j:2 * j + 2], st6[:])
                    std = p1mv.tile([128, GS], F32, tag="std")
                    nc.scalar.activation(std[:], mv[:, 1:2 * GS:2], AF.Sqrt,
                                         bias=epsb[:])
                    rr = p1mv.tile([128, GS], F32, tag="rr")
                    nc.vector.reciprocal(rr[:], std[:])
                    nmr = p1mv.tile([128, GS], F32, tag="nmr")
                    nc.vector.tensor_mul(nmr[:], mv[:, 0:2 * GS:2], rr[:])
                    nc.vector.tensor_scalar_mul(nmr[:], nmr[:], -1.0)
                    xn8 = p1xn.tile([128, GS, C], BF16, tag="xn8")
                    for j in range(GS):
                        if j < NORM_DVE_J:
                            nc.vector.tensor_scalar(
                                xn8[:, j, :], xt8[:, j, :], rr[:, j:j + 1],
                                nmr[:, j:j + 1], ALU.mult, ALU.add)
                        else:
                            nc.scalar.activation(
                                xn8[:, j, :], xt8[:, j, :], AF.Identity,
                                bias=nmr[:, j:j + 1], scale=rr[:, j:j + 1])
                    nc.sync.dma_start_transpose(
                        xT[:, g * GS * 128:(g + 1) * GS * 128].rearrange(
                            "c (s p) -> c s p", s=GS), xn8[:])

            # ================= Phase 1b: proj/gate/product ===============
            with tc.tile_pool(name="p1ps", bufs=2, space="PSUM") as p1ps, \
                 tc.tile_pool(name="p1sig", bufs=3) as p1sig, \
                 tc.tile_pool(name="p1pr", bufs=3) as p1pr:
                for q in range(2):
                    for g in range(NG):
                        for h2 in range(2):
                            pos0 = g * 2048 + h2 * 1024
                            ps = p1ps.tile([128, 2, 2, 512], F32, tag="ps")
                            for kind in range(2):
                                w0 = q * 256 + kind * 128
                                for hb in range(2):
                                    nc.tensor.matmul(
                                        ps[:, kind, hb, :],
                                        w1sb[:, w0:w0 + 128],
                                        xT[:, pos0 + hb * 512:
                                           pos0 + (hb + 1) * 512],
                                        start=True, stop=True)
                            sig = p1sig.tile([128, 2, 512], BF16, tag="sig")
                            nc.scalar.activation(sig[:], ps[:, 1, :, :],
                                                 AF.Sigmoid,
                                                 bias=c1sb[:, 2 * q + 1:
                                                           2 * q + 2])
                            prod = p1pr.tile([128, 2, 512], BF16, tag="prod")
                            nc.vector.scalar_tensor_tensor(
                                prod[:], ps[:, 0, :, :],
                                c1sb[:, 2 * q:2 * q + 1], sig[:],
                                ALU.add, ALU.mult)
                            if use_mask:
                                mrow = p1sig.tile([1, 1024], F32, tag="mrow")
                                nc.sync.dma_start(
                                    mrow[:], mask_r[pos0 // 1024:
                                                    pos0 // 1024 + 1, :])
                                mb = p1sig.tile([128, 1024], F32, tag="mb")
                                nc.gpsimd.partition_broadcast(mb[:], mrow[:])
                                nc.vector.tensor_mul(
                                    prod[:], prod[:],
                                    mb[:].rearrange("p (a b) -> p a b", a=2))
                            nc.sync.dma_start(
                                ab_i[q][:, pos0:pos0 + 1024],
                                prod[:].rearrange("p a b -> p (a b)"))
                    _collective([ab_i[q][:]], [ab_o[q][:]])

            # ================= Phase 2: einsum ===========================
            KC = N // 128
            with tc.tile_pool(name="p2a", bufs=2) as p2a, \
                 tc.tile_pool(name="p2b", bufs=2) as p2b, \
                 tc.tile_pool(name="p2o", bufs=4) as p2o, \
                 tc.tile_pool(name="p2mm", bufs=2, space="PSUM") as p2mm:
                for q in range(2):
                    for s in range(8):
                        AT = p2a.tile([128, KC, N], BF16, tag="AT")
                        BT = p2b.tile([128, KC, N], BF16, tag="BT")
                        for d in range(NCORES):
                            for which, Tt in ((0, AT), (1, BT)):
                                nc.sync.dma_start_transpose(
                                    Tt[:, :, d * R:(d + 1) * R],
                                    ab_o[q][16 * d + 2 * s + which,
                                            :].rearrange("(i k) -> i k",
                                                         i=R))
                        u = q * 8 + s
                        for ib in range(6):
                            ps2 = p2mm.tile([128, 2, 512], F32, tag="ps2")
                            for kc in range(KC):
                                lhsT = AT[:, kc, ib * 128:(ib + 1) * 128]
                                for jh in range(2):
                                    nc.tensor.matmul(
                                        ps2[:, jh, 0:384], lhsT,
                                        BT[:, kc, jh * 384:(jh + 1) * 384],
                                        start=(kc == 0), stop=(kc == KC - 1))
                            ot = p2o.tile([128, 2, 384], BF16, tag="ot")
                            if ib % 2 == 0:
                                nc.scalar.activation(ot[:], ps2[:, :, 0:384],
                                                     AF.Copy)
                            else:
                                nc.vector.tensor_copy(ot[:], ps2[:, :, 0:384])
                            otv = ot[:].rearrange("i h j -> i (h j)")
                            i0 = ib * 128
                            while i0 < (ib + 1) * 128:
                                d, off = divmod(i0, R)
                                n = min(R - off, (ib + 1) * 128 - i0)
                                nc.sync.dma_start(
                                    o2_i[16 * d + u, :].rearrange(
                                        "(i j) -> i j", i=R)[off:off + n, :],
                                    otv[i0 - ib * 128:i0 - ib * 128 + n, :])
                                i0 += n
                _collective([o2_i[:]], [o2_o[:]])

            # ================= Phase 3 ===================================
            with tc.tile_pool(name="p3a", bufs=2) as p3a, \
                 tc.tile_pool(name="p3st", bufs=2 * GS + 4) as p3st, \
                 tc.tile_pool(name="p3mv", bufs=1) as p3mv, \
                 tc.tile_pool(name="p3b", bufs=2) as p3b, \
                 tc.tile_pool(name="p3n", bufs=2) as p3n, \
                 tc.tile_pool(name="p3sig", bufs=2) as p3sig, \
                 tc.tile_pool(name="p3om", bufs=2) as p3om, \
                 tc.tile_pool(name="p3ps", bufs=2, space="PSUM") as p3ps:
                # passA: LN2 stats for all blocks
                mv3 = p3mv.tile([128, NG, 2 * GS], F32)
                for g in range(NG):
                    o2p = p3a.tile([128, GS, 128], BF16, tag="o2p")
                    nc.sync.dma_start_transpose(
                        o2p[:],
                        o2_o[:, g * GS * 128:(g + 1) * GS * 128])
                    for j in range(GS):
                        st6 = p3st.tile([128, 6], F32, tag="st63")
                        nc.vector.bn_stats(st6[:], o2p[:, j, :])
                        nc.vector.bn_aggr(mv3[:, g, 2 * j:2 * j + 2], st6[:])
                std3 = p3mv.tile([128, NG * GS], F32)
                nc.scalar.activation(
                    std3[:].rearrange("p (g s) -> p g s", g=NG),
                    mv3[:, :, 1:2 * GS:2], AF.Sqrt, bias=epsb[:])
                rr3 = p3mv.tile([128, NG * GS], F32)
                nc.vector.reciprocal(rr3[:], std3[:])
                nm3 = p3mv.tile([128, NG * GS], F32)
                nc.vector.tensor_mul(
                    nm3[:].rearrange("p (g s) -> p g s", g=NG),
                    mv3[:, :, 0:2 * GS:2],
                    rr3[:].rearrange("p (g s) -> p g s", g=NG))
                nc.vector.tensor_scalar_mul(nm3[:], nm3[:], -1.0)
                # passB: normalize + output matmuls
                for g in range(NG):
                    o2p = p3b.tile([128, GS, 128], BF16, tag="o2pb")
                    nc.sync.dma_start_transpose(
                        o2p[:],
                        o2_o[:, g * GS * 128:(g + 1) * GS * 128])
                    o2n = p3n.tile([128, GS, 128], BF16, tag="o2n")
                    for j in range(GS):
                        jj = g * GS + j
                        if j < NORM_DVE_J:
                            nc.vector.tensor_scalar(
                                o2n[:, j, :], o2p[:, j, :],
                                rr3[:, jj:jj + 1], nm3[:, jj:jj + 1],
                                ALU.mult, ALU.add)
                        else:
                            nc.scalar.activation(
                                o2n[:, j, :], o2p[:, j, :], AF.Identity,
                                bias=nm3[:, jj:jj + 1],
                                scale=rr3[:, jj:jj + 1])
                    o2nT = p3n.tile([128, GS, 128], BF16, tag="o2nT")
                    nc.sync.dma_start_transpose(o2nT[:], o2n[:])
                    for h2 in range(2):
                        pos0 = g * 2048 + h2 * 1024
                        o2v = o2nT[:].rearrange("c s p -> c (s p)")
                        ps3 = p3ps.tile([128, 2, 2, 512], F32, tag="ps3")
                        for hb in range(2):
                            sl = slice(h2 * 1024 + hb * 512,
                                       h2 * 1024 + (hb + 1) * 512)
                            nc.tensor.matmul(ps3[:, 0, hb, :],
                                             wfsb[:, 0:C], o2v[:, sl],
                                             start=True, stop=True)
                            nc.tensor.matmul(ps3[:, 1, hb, :],
                                             wfsb[:, C:2 * C],
                                             xT[:, pos0 + hb * 512:
                                                pos0 + (hb + 1) * 512],
                                             start=True, stop=True)
                        sigb = p3sig.tile([128, 2, 512], BF16, tag="sigb")
                        nc.scalar.activation(sigb[:], ps3[:, 1, :, :],
                                             AF.Sigmoid,
                                             bias=cfsb[:, 1:2])
                        om = p3om.tile([128, 1024], F32, tag="om")
                        nc.vector.scalar_tensor_tensor(
                            om[:].rearrange("p (a b) -> p a b", a=2),
                            ps3[:, 0, :, :], cfsb[:, 0:1], sigb[:],
                            ALU.add, ALU.mult)
                        nc.sync.dma_start(out_r[:, pos0:pos0 + 1024], om[:])
            if dbg:
                nc.sync.dma_start(dbg_xT[:, :], xT[:])
                nc.sync.dma_start(dbg_ab0i[:, :], ab_i[0][:, :])
                nc.sync.dma_start(dbg_o2i[:, :], o2_i[:, :])
    nc.compile()
    return nc


def _prep_weights(ln1_w, proj_w, gate_w, ln2_w, out_w, gating_w, ln1_b, ln2_b):
    p = np.arange(128)
    d, u = p // 16, p % 16
    w1 = np.zeros((C, 512), np.float32)
    c1 = np.zeros(512, np.float32)
    for q in range(2):
        c = d * 16 + q * 8 + u // 2
        row = 2 * c + (u % 2)
        w1[:, q * 256 + 0:q * 256 + 128] = (proj_w[row] * ln1_w).T
        w1[:, q * 256 + 128:q * 256 + 256] = (gate_w[row] * ln1_w).T
        c1[q * 256:q * 256 + 128] = proj_w[row] @ ln1_b
        c1[q * 256 + 128:q * 256 + 256] = gate_w[row] @ ln1_b
    wf = np.concatenate([(out_w * ln2_w[None, :]).T,
                         (gating_w * ln1_w[None, :]).T], axis=1)
    cf = np.concatenate([out_w @ ln2_b, gating_w @ ln1_b])
    # c1 layout for [128, 4] tile: index (p, 2q+kind) -> flat (2q+kind)*128+p
    c1t = c1.reshape(4, 128)  # rows: q0 proj, q0 gate, q1 proj, q1 gate
    c1t = np.stack([c1t[0], c1t[1], c1t[2], c1t[3]], 0).reshape(1, 512)
    cft = cf.reshape(1, 256)
    return w1, c1t, wf, cft


def kernel(pair, mask, ln1_w, ln1_b, proj_w, gate_w, ln2_w, ln2_b, out_w,
           gating_w):
    pair = np.asarray(pair, dtype=np.float32)
    mask = np.asarray(mask, dtype=np.float32)
    ln1_w = np.asarray(ln1_w, np.float32); ln1_b = np.asarray(ln1_b, np.float32)
    ln2_w = np.asarray(ln2_w, np.float32); ln2_b = np.asarray(ln2_b, np.float32)
    proj_w = np.asarray(proj_w, np.float32)
    gate_w = np.asarray(gate_w, np.float32)
    out_w = np.asarray(out_w, np.float32)
    gating_w = np.asarray(gating_w, np.float32)

    use_mask = not bool(np.all(mask == 1.0))
    if use_mask not in _PROGRAM_CACHE:
        _PROGRAM_CACHE[use_mask] = _build_program(use_mask)
    nc = _PROGRAM_CACHE[use_mask]

    w1, c1t, wf, cft = _prep_weights(ln1_w, proj_w, gate_w, ln2_w, out_w,
                                     gating_w, ln1_b, ln2_b)
    bf = ml_dtypes.bfloat16
    pair_b = pair.astype(bf).reshape(NCORES, POS, C)
    w1_b = np.ascontiguousarray(w1).astype(bf)
    wf_b = np.ascontiguousarray(wf).astype(bf)

    in_maps = []
    for c in range(NCORES):
        m = {
            "pair_r": pair_b[c],
            "w1t": w1_b,
            "wfin": wf_b,
            "c1r": c1t,
            "cfr": cft,
        }
        if use_mask:
            m["mask_r"] = np.ascontiguousarray(
                mask[c * R:(c + 1) * R].reshape(POS // 1024, 1024))
        in_maps.append(m)

    trace = os.environ.get("TRIMUL_TRACE", "") == "1"
    res = run_bass_kernel_spmd(nc, in_maps, core_ids=list(range(NCORES)),
                               trace=trace)
    global LAST_EXEC_NS, LAST_TRACE
    if res.exec_time_ns is not None:
        LAST_EXEC_NS = res.exec_time_ns
    if res.instructions_and_trace is not None:
        LAST_TRACE = res.instructions_and_trace[1]
    out = np.empty((N, N, C), np.float32)
    for c in range(NCORES):
        out[c * R:(c + 1) * R] = res.results[c]["out_r"].T.reshape(R, N, C)
    return out
